# revision 46
# baseline (speedup 1.0000x reference)
"""Multi-head attention (S=2048, B=2, D=1024, H=16) on 8 trn2 NeuronCores.

Sharding: 2 heads per core (head parallelism). Each core computes Q/K/V
projections for its 128 output features, attention for its 4 (batch,
head) pairs, and a partial output projection; the host sums the 8
partial outputs.

v4 restructure (from the 242us v3 trace): the TENSOR engine is the
bottleneck (~150us effective queue time), not the exp stream (~134us
busy), so the schedule keeps the tensor FIFO never-idle:
 - av matmuls and v_b/ej are bf16 (f32r moving operands streamed at
   half rate: 422ns -> 216ns per av matmul).
 - every 8-matmul projection group is split into 4+4 halves folded at
   adjacent slots so no fold bursts >1us delay the next scores pair
   (which gates exp via the 2-slot sj lead).
 - av(g) runs at slot g+1 (chunk 0: g+11, riding out the input DMA
   ramp) so an av never head-blocks the FIFO waiting on its exp.
 - consumption-ordered input DMA: k/q first-chunk halves, then k01..
   k03/v00..v03/q01 interleaved to match per-slot needs; weights ride
   the scalar HWDGE ring so the x stream owns the sync ring at t=0.
 - exp activation table preloaded at t~0 (dummy exp) to shave the
   2.7us ACT_TABLE_LOAD off the first real exp.
 - outputs: last two chunks on the (empty by then) sync HWDGE ring;
   earlier tiles trickle on the gpsimd SWDGE ring.
"""
import sys
sys.path.insert(0, '/opt/trn_rl_repo')
import functools
import os

import numpy as np

import concourse.bacc as bacc
import concourse.mybir as mybir
import concourse.tile as tile
from concourse.bass_utils import run_bass_kernel_spmd
from concourse.masks import make_identity

F32 = mybir.dt.float32
F32R = mybir.dt.float32r
F16 = mybir.dt.float16
BF16 = mybir.dt.bfloat16
AFT = mybir.ActivationFunctionType
MUL = mybir.AluOpType.mult

S, B, D, H = 2048, 2, 1024, 16
T = S * B               # 4096 tokens
DK = D // H             # 64
NC = 8                  # cores
FPC = D // NC           # 128 features per core (2 heads)
QC = 512                # q-chunk size
JT = S // 128           # 16 key tiles per batch
DT = D // 128           # 8 contraction tiles for projections
NCH = 8                 # chunks, b-outer: c -> b=c//4, qc=c%4
NSLOT = NCH * JT        # 128 global (chunk, key-tile) slots

EJ_DT = {"f32r": F32R, "bf16": BF16}[os.environ.get("EJ_DT", "bf16")]
OUT_DT = {"fp16": F16, "fp32": F32}[os.environ.get("OUT_DT", "fp16")]


def build_nc():
    nc = bacc.Bacc(None, target_bir_lowering=False)

    # x pre-arranged host-side as [128, (b hf) a m]: every 512-token tile is
    # contiguous per partition -> 128 DMA descriptors per tile, not 1024.
    XW = B * 4 * DT * 512
    xq = nc.dram_tensor("xq", [128, XW], F16, kind="ExternalInput")
    xk = nc.dram_tensor("xk", [128, XW], F16, kind="ExternalInput")
    xv = nc.dram_tensor("xv", [128, XW], F16, kind="ExternalInput")
    wq = nc.dram_tensor("wq", [128, DT * FPC], F16, kind="ExternalInput")
    wk = nc.dram_tensor("wk", [128, DT * FPC], F16, kind="ExternalInput")
    wv = nc.dram_tensor("wv", [128, DT * FPC], F16, kind="ExternalInput")
    wo = nc.dram_tensor("wo", [FPC, D], F16, kind="ExternalInput")
    out = nc.dram_tensor("out", [T, D], OUT_DT, kind="ExternalOutput")
    xsrc = {"q": xq, "k": xk, "v": xv}

    with tile.TileContext(nc) as tc:
        with (
            tc.tile_pool(name="wpool", bufs=1) as wpool,
            tc.tile_pool(name="proj", bufs=1) as projpool,
            tc.tile_pool(name="vtmp", bufs=2) as vtpool,
            tc.tile_pool(name="xdma", bufs=8) as xpool,
            tc.tile_pool(name="ej", bufs=26) as epool,
            tc.tile_pool(name="norm", bufs=2) as npool,
            tc.tile_pool(name="osb", bufs=3) as opool,
            tc.tile_pool(name="psS", bufs=2, space="PSUM") as psS,
            tc.tile_pool(name="psA", bufs=2, space="PSUM") as psA,
            tc.tile_pool(name="psM", bufs=2, space="PSUM") as psM,
        ):
            # ---- exp table preload: dummy exp at t~0 so the first real
            # exp doesn't pay the ~2.7us ACT_TABLE_LOAD.
            warm = npool.tile([1, 8], F32, name="warm", tag="warm")
            warm_o = npool.tile([1, 8], F32, name="warm_o", tag="warm_o")
            nc.vector.memset(warm[:], 0.0)
            nc.scalar.activation(warm_o[:], warm[:], AFT.Exp)

            # ---- weights / constants ----
            # pre-packed [128, DT*FPC] host-side: contiguous per partition;
            # scalar HWDGE ring so the x stream owns sync. Emission order on
            # the scalar ring is interleaved with the first k00/q00 half
            # tiles below (wk -> k00b -> wq -> q00b -> wv).
            w_t = {}
            for name, wd in (("k", wk), ("q", wq), ("v", wv)):
                w_t[name] = wpool.tile([128, DT, FPC], F16, name=f"w_{name}")
            nc.scalar.dma_start(w_t["k"][:], wk.rearrange("p (t m) -> p t m", t=DT))
            ident = wpool.tile([128, 128], BF16, name="ident")
            make_identity(nc, ident[:])
            wo_t = wpool.tile([128, D], F16, name="wo_t")

            # ---- persistent activations ----
            kT = [projpool.tile([128, S], F16, name=f"kT{b}") for b in range(B)]
            qT = [projpool.tile([128, S], F16, name=f"qT{b}") for b in range(B)]
            v_b = [projpool.tile([128, JT, 130], BF16, name=f"v_b{b}") for b in range(B)]
            xT = projpool.tile([128, T], F16, name="xT")
            # ones columns of v_b (denominator trick) are static
            for b in range(B):
                for jt in range(JT):
                    nc.vector.memset(v_b[b][:, jt, 64:65], 1.0)
                    nc.vector.memset(v_b[b][:, jt, 129:130], 1.0)

            # ---- input DMA stream (emission order == transfer order) ----
            xt_tiles = {}
            xh_tiles = {}

            def emit_xdma(p, b, hf):
                t = xpool.tile([128, DT, 512], F16, name="xt", tag="xt")
                idx = b * 4 + hf
                nc.sync.dma_start(
                    t[:],
                    xsrc[p][:, idx * DT * 512:(idx + 1) * DT * 512]
                    .rearrange("p (a m) -> p a m", a=DT))
                xt_tiles[(p, b, hf)] = t

            def emit_xdma_split(p, b, hf):
                # first tiles: dt-halves split across BOTH HWDGE rings so the
                # early DMA ramp runs in parallel; the proj's first 4 matmuls
                # (dt 0-3) only depend on the sync half.
                t = xpool.tile([128, DT, 512], F16, name="xt", tag="xt")
                base = (b * 4 + hf) * DT * 512
                nc.sync.dma_start(
                    t[:, 0:DT // 2, :],
                    xsrc[p][:, base:base + DT * 256]
                    .rearrange("p (a m) -> p a m", a=DT // 2))
                nc.scalar.dma_start(
                    t[:, DT // 2:DT, :],
                    xsrc[p][:, base + DT * 256:base + DT * 512]
                    .rearrange("p (a m) -> p a m", a=DT // 2))
                xt_tiles[(p, b, hf)] = t

            emit_xdma_split("k", 0, 0)
            nc.scalar.dma_start(w_t["q"][:], wq.rearrange("p (t m) -> p t m", t=DT))
            emit_xdma_split("q", 0, 0)
            nc.scalar.dma_start(w_t["v"][:], wv.rearrange("p (t m) -> p t m", t=DT))
            dma_order = [
                ("k", 0, 1), ("v", 0, 0), ("k", 0, 2), ("v", 0, 1), ("k", 0, 3),
                ("v", 0, 2), ("q", 0, 1), ("v", 0, 3),
                "WO",
                ("q", 0, 2), ("q", 0, 3), ("k", 1, 0), ("k", 1, 1), ("k", 1, 2),
                ("k", 1, 3), ("v", 1, 0), ("v", 1, 1), ("v", 1, 2), ("v", 1, 3),
                ("q", 1, 0), ("q", 1, 1), ("q", 1, 2), ("q", 1, 3),
            ]
            for item in dma_order:
                if item == "WO":
                    nc.sync.dma_start(wo_t[:], wo[:, :])
                else:
                    emit_xdma(*item)

            # ---- building blocks ----
            proj_ps = {}

            def proj_mm_half(p, b, hf, dest, part, evac_scalar=False):
                """half (4 matmuls) of an 8-mm accumulation over 512 tokens."""
                key = (p, b, hf)
                if part == 0:
                    proj_ps[key] = psM.tile([128, 512], F32, name="psproj", tag="m")
                ps = proj_ps[key]
                xt = xt_tiles[key]
                for dt in range(4 * part, 4 * part + 4):
                    nc.tensor.matmul(ps[:], w_t[p][:, dt, :], xt[:, dt, :],
                                     start=(dt == 0), stop=(dt == DT - 1))
                if part == 1:
                    del xt_tiles[key], proj_ps[key]
                    if evac_scalar:
                        # qT evacs gate the next chunk's scores; the vector
                        # queue bursts (op CASTs + normalize) at chunk
                        # boundaries delay them ~2-6us. The scalar queue
                        # dispatches this between exps at ~zero delay.
                        nc.scalar.copy(dest, ps[:])
                    else:
                        nc.vector.tensor_copy(dest, ps[:])

            vstore = {}

            def v_proj_half(b, hf, part):
                if part == 0:
                    vstore[(b, hf)] = vtpool.tile([128, 512], BF16, name="vt", tag="vt")
                proj_mm_half("v", b, hf, vstore[(b, hf)][:], part)

            def v_tp(b, hf, jj):
                """transpose 2 of the 4 key-tiles of v half hf into v_b."""
                vt = vstore[(b, hf)]
                for u in range(2):
                    q = 2 * jj + u
                    jt = 4 * hf + q
                    tp = psM.tile([128, 128], BF16, name="tp", tag="m")
                    nc.tensor.transpose(
                        tp[:], vt[:, q * 128:(q + 1) * 128], ident[:])
                    nc.vector.tensor_copy(v_b[b][:, jt, 0:64], tp[:, 0:64])
                    nc.vector.tensor_copy(v_b[b][:, jt, 65:129], tp[:, 64:128])

            sj_t = {}
            ej_t = {}

            def emit_scores(g):
                c, j = divmod(g, JT)
                b, qc = divmod(c, 4)
                sj = psS.tile([128, 2, QC], F32, name="sj", tag="sj")
                for h in range(2):
                    nc.tensor.matmul(
                        sj[:, h, :], kT[b][h * 64:(h + 1) * 64, j * 128:(j + 1) * 128],
                        qT[b][h * 64:(h + 1) * 64, qc * QC:(qc + 1) * QC],
                        start=True, stop=True)
                sj_t[g] = sj

            def emit_exp(g):
                ej = epool.tile([128, 2, QC], EJ_DT, name="ej", tag="ej")
                nc.scalar.activation(ej[:], sj_t.pop(g)[:], AFT.Exp)
                ej_t[g] = ej

            def emit_av(g, pacc):
                c, j = divmod(g, JT)
                b = c // 4
                ej = ej_t.pop(g)
                for h in range(2):
                    nc.tensor.matmul(
                        pacc[h][0:65, :], v_b[b][:, j, h * 65:h * 65 + 65],
                        ej[:, h, :], start=(j == 0), stop=(j == JT - 1))

            def emit_normalize_h(c, pacc, h):
                dsb = npool.tile([1, QC], F32, name="dsb", tag="dsb")
                # for the last chunk the scalar engine is idle (exp done) —
                # use it for the PSUM read so h0/h1 chains overlap
                if c == NCH - 1:
                    nc.scalar.copy(dsb[:], pacc[h][64:65, :])
                else:
                    nc.vector.tensor_copy(dsb[:], pacc[h][64:65, :])
                rd = npool.tile([1, QC], F32, name="rd", tag="rd")
                nc.vector.reciprocal_approx_fast(out=rd[:], in_=dsb[:])
                bc = npool.tile([64, QC], F32, name="bc", tag="bc")
                nc.gpsimd.partition_broadcast(bc[:], rd[:])
                nc.vector.tensor_tensor(
                    out=xT[h * 64:(h + 1) * 64, c * QC:(c + 1) * QC],
                    in0=pacc[h][0:64, :], in1=bc[:], op=MUL)

            def emit_oproj(tt, use_act=False):
                osb = opool.tile([128, D], OUT_DT, name="osb", tag="osb")
                for ec in range(2):
                    po = psM.tile([128, 512], F32, name="po", tag="m")
                    nc.tensor.matmul(po[:], xT[:, tt * 128:(tt + 1) * 128],
                                     wo_t[:, ec * 512:(ec + 1) * 512],
                                     start=True, stop=True)
                    if use_act and ec == 0:
                        nc.scalar.copy(osb[:, 0:512], po[:])
                    else:
                        nc.vector.tensor_copy(osb[:, ec * 512:(ec + 1) * 512], po[:])
                # last two chunks' outputs ride the (empty by then) sync
                # HWDGE ring at full rate; earlier tiles trickle on SWDGE.
                eng = nc.sync if tt >= 24 else nc.gpsimd
                eng.dma_start(out[tt * 128:(tt + 1) * 128, :], osb[:])

            # ---- fold schedule (each item <=~1us of PE) ----
            def k_half(hf, part):
                return lambda: proj_mm_half(
                    "k", 0, hf, kT[0][:, hf * 512:(hf + 1) * 512], part)

            def k1_half(hf, part):
                return lambda: proj_mm_half(
                    "k", 1, hf, kT[1][:, hf * 512:(hf + 1) * 512], part)

            def q_half(c, part):
                b, qc = divmod(c, 4)
                return lambda: proj_mm_half(
                    "q", b, qc, qT[b][:, qc * 512:(qc + 1) * 512], part,
                    evac_scalar=(c >= 1))

            def vp_half(b, hf, part):
                return lambda: v_proj_half(b, hf, part)

            def vt_item(b, hf, jj):
                return lambda: v_tp(b, hf, jj)

            def op_item(tt):
                return lambda: emit_oproj(tt)

            FOLD = {}

            def put(g, th):
                FOLD.setdefault(g, []).append(th)

            def put_pair(g0, mk, *args):
                put(g0, mk(*args, 0))
                put(g0 + 1, mk(*args, 1))

            # b0 projections + v prep, arrival-matched to the DMA stream
            put_pair(0, k_half, 1)
            put_pair(2, vp_half, 0, 0)
            put_pair(4, k_half, 2)
            put(6, vt_item(0, 0, 0)); put(6, vt_item(0, 0, 1))
            put_pair(7, vp_half, 0, 1)
            put_pair(8, k_half, 3)
            put(11, vt_item(0, 1, 0)); put(11, vt_item(0, 1, 1))
            put_pair(12, q_half, 1)
            put_pair(14, vp_half, 0, 2)
            put(16, vt_item(0, 2, 0)); put(16, vt_item(0, 2, 1))
            put_pair(17, vp_half, 0, 3)
            put(19, vt_item(0, 3, 0)); put(19, vt_item(0, 3, 1))
            put_pair(26, q_half, 2)
            # b1 prep under b0's chunks 2-3; q03 first — its deadline
            # (scores(48) emitted at slot 46) has no slack, k1x's does.
            put_pair(34, q_half, 3)
            put_pair(44, k1_half, 0)
            put_pair(46, k1_half, 1)
            put_pair(48, k1_half, 2)
            put_pair(50, k1_half, 3)
            put_pair(52, q_half, 4)
            put_pair(54, vp_half, 1, 0)
            put(56, vt_item(1, 0, 0)); put(57, vt_item(1, 0, 1))
            put_pair(58, vp_half, 1, 1)
            put(60, vt_item(1, 1, 0)); put(61, vt_item(1, 1, 1))
            put_pair(62, vp_half, 1, 2)
            put(64, vt_item(1, 2, 0)); put(65, vt_item(1, 2, 1))
            put_pair(66, vp_half, 1, 3)
            put(68, vt_item(1, 3, 0)); put(69, vt_item(1, 3, 1))
            put_pair(74, q_half, 5)
            put_pair(90, q_half, 6)
            put_pair(106, q_half, 7)
            # output projection tiles, spread into fold-free slots
            op_slots = {0: (29, 31, 33, 42), 1: (70, 72, 76, 78),
                        2: (80, 82, 84, 86), 3: (88, 92, 94, 96),
                        4: (98, 100, 102, 104), 5: (108, 110, 112, 114),
                        6: (116, 118, 120, 121)}
            for c, slots in op_slots.items():
                for i, g in enumerate(slots):
                    put(g, op_item(4 * c + i))

            # per-chunk av lag: chunk 0 rides out the input DMA ramp
            AV_LAG = {c: (11 if c == 0 else 1) for c in range(NCH)}
            AV_AT = {}
            for g in range(NSLOT):
                AV_AT.setdefault(g + AV_LAG[g // JT], []).append(g)

            # ---- prologue: first k/q projections ----
            for part in (0, 1):
                proj_mm_half("k", 0, 0, kT[0][:, 0:512], part)
            for part in (0, 1):
                proj_mm_half("q", 0, 0, qT[0][:, 0:512], part)
            emit_scores(0)
            emit_scores(1)

            # ---- the stream ----
            paccs = {}

            norm_pend = {}

            def run_avs(slot):
                # second normalize half from the previous chunk, one slot
                # later, so the vector burst at a chunk boundary is halved
                for ca, pacc in list(norm_pend.items()):
                    emit_normalize_h(ca, pacc, 1)
                    del norm_pend[ca]
                for ga in AV_AT.get(slot, ()):
                    ca = ga // JT
                    if ga % JT == 0:
                        paccs[ca] = [psA.tile([128, QC], F32, name=f"pacc{h}",
                                              tag="pacc") for h in range(2)]
                    emit_av(ga, paccs[ca])
                    if ga % JT == JT - 1:
                        pacc = paccs.pop(ca)
                        emit_normalize_h(ca, pacc, 0)
                        norm_pend[ca] = pacc

            for g in range(NSLOT):
                emit_exp(g)
                if g + 2 < NSLOT:
                    emit_scores(g + 2)
                for th in FOLD.get(g, ()):
                    th()
                run_avs(g)

            # ---- tail: trailing avs, then last chunk's output projection ----
            for slot in range(NSLOT, NSLOT + max(AV_LAG.values()) + 1):
                run_avs(slot)
            for tt in range(28, 32):
                emit_oproj(tt, use_act=True)
    nc.finalize()
    return nc


@functools.cache
def _nc_cached():
    return build_nc()


def _prep_in_maps(inputs):
    np16 = np.float16

    def xbm(a):
        # [S,B,D] -> [128, (b hf) a m]: tile-contiguous per partition
        xd = np.asarray(a, np.float32).transpose(2, 1, 0)        # [D, B, S]
        xd = xd.reshape(DT, 128, B, 4, 512).transpose(1, 2, 3, 0, 4)
        return np.ascontiguousarray(xd.reshape(128, B * 4 * DT * 512)).astype(np16)

    xq_h = xbm(inputs["query"])
    xk_h = xbm(inputs["key"])
    xv_h = xbm(inputs["value"])
    Wq, Wk, Wv, Wo = (np.asarray(inputs[k], np.float32) for k in ("Wq", "Wk", "Wv", "Wo"))

    def wpack(w):
        # [D, FPC] (t p)-major -> [p=128, t*m] contiguous per partition
        return np.ascontiguousarray(
            w.reshape(DT, 128, FPC).transpose(1, 0, 2).reshape(128, DT * FPC)
        ).astype(np16)

    in_maps = []
    for c in range(NC):
        sl = slice(c * FPC, (c + 1) * FPC)
        in_maps.append({
            "xq": xq_h, "xk": xk_h, "xv": xv_h,
            "wq": wpack(Wq[sl, :].T),
            "wk": wpack(Wk[sl, :].T),
            "wv": wpack(Wv[sl, :].T),
            "wo": np.ascontiguousarray(Wo[:, sl].T).astype(np16),
        })
    return in_maps


def kernel(query, key, value, Wq, bq, Wk, bk, Wv, bv, Wo, bo):
    in_maps = _prep_in_maps({"query": query, "key": key, "value": value,
                             "Wq": Wq, "Wk": Wk, "Wv": Wv, "Wo": Wo})
    nc = _nc_cached()
    res = run_bass_kernel_spmd(nc, in_maps, core_ids=list(range(NC)))
    acc = np.zeros((T, D), np.float32)
    for r in res.results:
        acc += r["out"].astype(np.float32)
    acc += np.asarray(bo, np.float32)[None, :]
    # batch-major tokens back to [S, B, D]
    out = acc.reshape(B, S, D).transpose(1, 0, 2)
    out = np.ascontiguousarray(out, np.float32)
    for bias in (bq, bk, bv):
        assert float(np.abs(np.asarray(bias)).max()) == 0.0, "nonzero qkv bias unsupported"
    return out


# revision 49
# speedup vs baseline: 1.0333x; 1.0333x over previous
"""Multi-head attention (S=2048, B=2, D=1024, H=16) on 8 trn2 NeuronCores.

Sharding: 2 heads per core (head parallelism). Each core computes Q/K/V
projections for its 128 output features, attention for its 4 (batch,
head) pairs, and a partial output projection; the host sums the 8
partial outputs.

v4 restructure (from the 242us v3 trace): the TENSOR engine is the
bottleneck (~150us effective queue time), not the exp stream (~134us
busy), so the schedule keeps the tensor FIFO never-idle:
 - av matmuls and v_b/ej are bf16 (f32r moving operands streamed at
   half rate: 422ns -> 216ns per av matmul).
 - every 8-matmul projection group is split into 4+4 halves folded at
   adjacent slots so no fold bursts >1us delay the next scores pair
   (which gates exp via the 2-slot sj lead).
 - av(g) runs at slot g+1 (chunk 0: g+11, riding out the input DMA
   ramp) so an av never head-blocks the FIFO waiting on its exp.
 - consumption-ordered input DMA: k/q first-chunk halves, then k01..
   k03/v00..v03/q01 interleaved to match per-slot needs; weights ride
   the scalar HWDGE ring so the x stream owns the sync ring at t=0.
 - exp activation table preloaded at t~0 (dummy exp) to shave the
   2.7us ACT_TABLE_LOAD off the first real exp.
 - outputs: last two chunks on the (empty by then) sync HWDGE ring;
   earlier tiles trickle on the gpsimd SWDGE ring.
"""
import sys
sys.path.insert(0, '/opt/trn_rl_repo')
import functools
import os

import numpy as np

import concourse.bacc as bacc
import concourse.mybir as mybir
import concourse.tile as tile
from concourse.bass_utils import run_bass_kernel_spmd
from concourse.masks import make_identity

F32 = mybir.dt.float32
F32R = mybir.dt.float32r
F16 = mybir.dt.float16
BF16 = mybir.dt.bfloat16
AFT = mybir.ActivationFunctionType
MUL = mybir.AluOpType.mult

S, B, D, H = 2048, 2, 1024, 16
T = S * B               # 4096 tokens
DK = D // H             # 64
NC = 8                  # cores
FPC = D // NC           # 128 features per core (2 heads)
QC = 512                # q-chunk size
JT = S // 128           # 16 key tiles per batch
DT = D // 128           # 8 contraction tiles for projections
NCH = 8                 # chunks, b-outer: c -> b=c//4, qc=c%4
NSLOT = NCH * JT        # 128 global (chunk, key-tile) slots

EJ_DT = {"f32r": F32R, "bf16": BF16}[os.environ.get("EJ_DT", "bf16")]
OUT_DT = {"fp16": F16, "fp32": F32}[os.environ.get("OUT_DT", "fp16")]


def build_nc():
    nc = bacc.Bacc(None, target_bir_lowering=False)

    # x pre-arranged host-side as [128, (b hf) a m]: every 512-token tile is
    # contiguous per partition -> 128 DMA descriptors per tile, not 1024.
    XW = B * 4 * DT * 512
    xq = nc.dram_tensor("xq", [128, XW], F16, kind="ExternalInput")
    xk = nc.dram_tensor("xk", [128, XW], F16, kind="ExternalInput")
    xv = nc.dram_tensor("xv", [128, XW], F16, kind="ExternalInput")
    wq = nc.dram_tensor("wq", [128, DT * FPC], F16, kind="ExternalInput")
    wk = nc.dram_tensor("wk", [128, DT * FPC], F16, kind="ExternalInput")
    wv = nc.dram_tensor("wv", [128, DT * FPC], F16, kind="ExternalInput")
    wo = nc.dram_tensor("wo", [FPC, D], F16, kind="ExternalInput")
    out = nc.dram_tensor("out", [T, D], OUT_DT, kind="ExternalOutput")
    xsrc = {"q": xq, "k": xk, "v": xv}

    with tile.TileContext(nc) as tc:
        with (
            tc.tile_pool(name="wpool", bufs=1) as wpool,
            tc.tile_pool(name="proj", bufs=1) as projpool,
            tc.tile_pool(name="vtmp", bufs=2) as vtpool,
            tc.tile_pool(name="xdma", bufs=8) as xpool,
            tc.tile_pool(name="ej", bufs=26) as epool,
            tc.tile_pool(name="norm", bufs=2) as npool,
            tc.tile_pool(name="osb", bufs=3) as opool,
            tc.tile_pool(name="psS", bufs=2, space="PSUM") as psS,
            tc.tile_pool(name="psA", bufs=2, space="PSUM") as psA,
            tc.tile_pool(name="psM", bufs=2, space="PSUM") as psM,
        ):
            # ---- exp table preload: dummy exp at t~0 so the first real
            # exp doesn't pay the ~2.7us ACT_TABLE_LOAD.
            warm = npool.tile([1, 8], F32, name="warm", tag="warm")
            warm_o = npool.tile([1, 8], F32, name="warm_o", tag="warm_o")
            nc.vector.memset(warm[:], 0.0)
            nc.scalar.activation(warm_o[:], warm[:], AFT.Exp)

            # ---- weights / constants ----
            w_t = {}
            for name, wd in (("k", wk), ("q", wq), ("v", wv)):
                w_t[name] = wpool.tile([128, DT, FPC], F16, name=f"w_{name}")
                # pre-packed [128, DT*FPC] host-side: contiguous per
                # partition; scalar HWDGE ring so the x stream owns sync.
                nc.scalar.dma_start(w_t[name][:], wd.rearrange("p (t m) -> p t m", t=DT))
            ident = wpool.tile([128, 128], BF16, name="ident")
            make_identity(nc, ident[:])
            wo_t = wpool.tile([128, D], F16, name="wo_t")

            # ---- persistent activations ----
            kT = [projpool.tile([128, S], F16, name=f"kT{b}") for b in range(B)]
            qT = [projpool.tile([128, S], F16, name=f"qT{b}") for b in range(B)]
            v_b = [projpool.tile([128, JT, 130], BF16, name=f"v_b{b}") for b in range(B)]
            xT = projpool.tile([128, T], F16, name="xT")
            # ones columns of v_b (denominator trick) are static
            for b in range(B):
                for jt in range(JT):
                    nc.vector.memset(v_b[b][:, jt, 64:65], 1.0)
                    nc.vector.memset(v_b[b][:, jt, 129:130], 1.0)

            # ---- input DMA stream (emission order == transfer order) ----
            xt_tiles = {}
            xh_tiles = {}

            def emit_xdma(p, b, hf):
                t = xpool.tile([128, DT, 512], F16, name="xt", tag="xt")
                idx = b * 4 + hf
                nc.sync.dma_start(
                    t[:],
                    xsrc[p][:, idx * DT * 512:(idx + 1) * DT * 512]
                    .rearrange("p (a m) -> p a m", a=DT))
                xt_tiles[(p, b, hf)] = t

            emit_xdma("k", 0, 0)
            emit_xdma("q", 0, 0)
            dma_order = [
                ("k", 0, 1), ("v", 0, 0), ("k", 0, 2), ("v", 0, 1), ("k", 0, 3),
                ("v", 0, 2), ("q", 0, 1), ("v", 0, 3),
                "WO",
                ("q", 0, 2), ("q", 0, 3), ("k", 1, 0), ("k", 1, 1), ("k", 1, 2),
                ("k", 1, 3), ("v", 1, 0), ("v", 1, 1), ("v", 1, 2), ("v", 1, 3),
                ("q", 1, 0), ("q", 1, 1), ("q", 1, 2), ("q", 1, 3),
            ]
            for item in dma_order:
                if item == "WO":
                    nc.sync.dma_start(wo_t[:], wo[:, :])
                else:
                    emit_xdma(*item)

            # ---- building blocks ----
            proj_ps = {}

            def proj_mm_half(p, b, hf, dest, part, evac_scalar=False):
                """half (4 matmuls) of an 8-mm accumulation over 512 tokens."""
                key = (p, b, hf)
                if part == 0:
                    proj_ps[key] = psM.tile([128, 512], F32, name="psproj", tag="m")
                ps = proj_ps[key]
                xt = xt_tiles[key]
                for dt in range(4 * part, 4 * part + 4):
                    nc.tensor.matmul(ps[:], w_t[p][:, dt, :], xt[:, dt, :],
                                     start=(dt == 0), stop=(dt == DT - 1))
                if part == 1:
                    del xt_tiles[key], proj_ps[key]
                    if evac_scalar:
                        # qT evacs gate the next chunk's scores; the vector
                        # queue bursts (op CASTs + normalize) at chunk
                        # boundaries delay them ~2-6us. The scalar queue
                        # dispatches this between exps at ~zero delay.
                        nc.scalar.copy(dest, ps[:])
                    else:
                        nc.vector.tensor_copy(dest, ps[:])

            vstore = {}

            def v_proj_half(b, hf, part):
                if part == 0:
                    vstore[(b, hf)] = vtpool.tile([128, 512], BF16, name="vt", tag="vt")
                proj_mm_half("v", b, hf, vstore[(b, hf)][:], part)

            def v_tp(b, hf, jj):
                """transpose 2 of the 4 key-tiles of v half hf into v_b."""
                vt = vstore[(b, hf)]
                for u in range(2):
                    q = 2 * jj + u
                    jt = 4 * hf + q
                    tp = psM.tile([128, 128], BF16, name="tp", tag="m")
                    nc.tensor.transpose(
                        tp[:], vt[:, q * 128:(q + 1) * 128], ident[:])
                    nc.vector.tensor_copy(v_b[b][:, jt, 0:64], tp[:, 0:64])
                    nc.vector.tensor_copy(v_b[b][:, jt, 65:129], tp[:, 64:128])

            sj_t = {}
            ej_t = {}

            def emit_scores(g):
                c, j = divmod(g, JT)
                b, qc = divmod(c, 4)
                sj = psS.tile([128, 2, QC], F32, name="sj", tag="sj")
                for h in range(2):
                    nc.tensor.matmul(
                        sj[:, h, :], kT[b][h * 64:(h + 1) * 64, j * 128:(j + 1) * 128],
                        qT[b][h * 64:(h + 1) * 64, qc * QC:(qc + 1) * QC],
                        start=True, stop=True)
                sj_t[g] = sj

            def emit_exp(g):
                ej = epool.tile([128, 2, QC], EJ_DT, name="ej", tag="ej")
                nc.scalar.activation(ej[:], sj_t.pop(g)[:], AFT.Exp)
                ej_t[g] = ej

            def emit_av(g, pacc):
                c, j = divmod(g, JT)
                b = c // 4
                ej = ej_t.pop(g)
                for h in range(2):
                    nc.tensor.matmul(
                        pacc[h][0:65, :], v_b[b][:, j, h * 65:h * 65 + 65],
                        ej[:, h, :], start=(j == 0), stop=(j == JT - 1))

            def emit_normalize_h(c, pacc, h):
                dsb = npool.tile([1, QC], F32, name="dsb", tag="dsb")
                # for the last chunk the scalar engine is idle (exp done) —
                # use it for the PSUM read so h0/h1 chains overlap
                if c == NCH - 1:
                    nc.scalar.copy(dsb[:], pacc[h][64:65, :])
                else:
                    nc.vector.tensor_copy(dsb[:], pacc[h][64:65, :])
                rd = npool.tile([1, QC], F32, name="rd", tag="rd")
                nc.vector.reciprocal_approx_fast(out=rd[:], in_=dsb[:])
                bc = npool.tile([64, QC], F32, name="bc", tag="bc")
                nc.gpsimd.partition_broadcast(bc[:], rd[:])
                if c == NCH - 1:
                    # tail: 128-col pieces so each oproj tile starts on its
                    # own piece instead of the full 512-col multiply
                    for u in range(4):
                        s = slice(u * 128, (u + 1) * 128)
                        nc.vector.tensor_tensor(
                            out=xT[h * 64:(h + 1) * 64, c * QC + u * 128:
                                   c * QC + (u + 1) * 128],
                            in0=pacc[h][0:64, s], in1=bc[:, s], op=MUL)
                else:
                    nc.vector.tensor_tensor(
                        out=xT[h * 64:(h + 1) * 64, c * QC:(c + 1) * QC],
                        in0=pacc[h][0:64, :], in1=bc[:], op=MUL)

            def emit_oproj(tt, use_act=False):
                osb = opool.tile([128, D], OUT_DT, name="osb", tag="osb")
                for ec in range(2):
                    po = psM.tile([128, 512], F32, name="po", tag="m")
                    nc.tensor.matmul(po[:], xT[:, tt * 128:(tt + 1) * 128],
                                     wo_t[:, ec * 512:(ec + 1) * 512],
                                     start=True, stop=True)
                    if use_act and ec == 0:
                        nc.scalar.copy(osb[:, 0:512], po[:])
                    else:
                        nc.vector.tensor_copy(osb[:, ec * 512:(ec + 1) * 512], po[:])
                # last two chunks' outputs ride the (empty by then) sync
                # HWDGE ring at full rate; earlier tiles trickle on SWDGE.
                eng = nc.sync if tt >= 24 else nc.gpsimd
                eng.dma_start(out[tt * 128:(tt + 1) * 128, :], osb[:])

            # ---- fold schedule (each item <=~1us of PE) ----
            def k_half(hf, part):
                return lambda: proj_mm_half(
                    "k", 0, hf, kT[0][:, hf * 512:(hf + 1) * 512], part)

            def k1_half(hf, part):
                return lambda: proj_mm_half(
                    "k", 1, hf, kT[1][:, hf * 512:(hf + 1) * 512], part)

            def q_half(c, part):
                b, qc = divmod(c, 4)
                return lambda: proj_mm_half(
                    "q", b, qc, qT[b][:, qc * 512:(qc + 1) * 512], part,
                    evac_scalar=(c >= 1))

            def vp_half(b, hf, part):
                return lambda: v_proj_half(b, hf, part)

            def vt_item(b, hf, jj):
                return lambda: v_tp(b, hf, jj)

            def op_item(tt):
                return lambda: emit_oproj(tt)

            FOLD = {}

            def put(g, th):
                FOLD.setdefault(g, []).append(th)

            def put_pair(g0, mk, *args):
                put(g0, mk(*args, 0))
                put(g0 + 1, mk(*args, 1))

            # b0 projections + v prep, arrival-matched to the DMA stream
            put_pair(0, k_half, 1)
            put_pair(2, vp_half, 0, 0)
            put_pair(4, k_half, 2)
            put(6, vt_item(0, 0, 0)); put(6, vt_item(0, 0, 1))
            put_pair(7, vp_half, 0, 1)
            put_pair(8, k_half, 3)
            put(11, vt_item(0, 1, 0)); put(11, vt_item(0, 1, 1))
            put_pair(12, q_half, 1)
            put_pair(14, vp_half, 0, 2)
            put(16, vt_item(0, 2, 0)); put(16, vt_item(0, 2, 1))
            put_pair(17, vp_half, 0, 3)
            put(19, vt_item(0, 3, 0)); put(19, vt_item(0, 3, 1))
            put_pair(26, q_half, 2)
            # b1 prep under b0's chunks 2-3; q03 first — its deadline
            # (scores(48) emitted at slot 46) has no slack, k1x's does.
            put_pair(34, q_half, 3)
            put_pair(44, k1_half, 0)
            put_pair(46, k1_half, 1)
            put_pair(48, k1_half, 2)
            put_pair(50, k1_half, 3)
            put_pair(52, q_half, 4)
            put_pair(54, vp_half, 1, 0)
            put(56, vt_item(1, 0, 0)); put(57, vt_item(1, 0, 1))
            put_pair(58, vp_half, 1, 1)
            put(60, vt_item(1, 1, 0)); put(61, vt_item(1, 1, 1))
            put_pair(62, vp_half, 1, 2)
            put(64, vt_item(1, 2, 0)); put(65, vt_item(1, 2, 1))
            put_pair(66, vp_half, 1, 3)
            put(68, vt_item(1, 3, 0)); put(69, vt_item(1, 3, 1))
            put_pair(74, q_half, 5)
            put_pair(90, q_half, 6)
            put_pair(106, q_half, 7)
            # output projection tiles, spread into fold-free slots
            op_slots = {0: (29, 31, 33, 42), 1: (70, 72, 76, 78),
                        2: (80, 82, 84, 86), 3: (88, 92, 94, 96),
                        4: (98, 100, 102, 104), 5: (108, 110, 112, 114),
                        6: (116, 118, 120, 121)}
            for c, slots in op_slots.items():
                for i, g in enumerate(slots):
                    put(g, op_item(4 * c + i))

            # per-chunk av lag: chunk 0 rides out the input DMA ramp
            AV_LAG = {c: (11 if c == 0 else 1) for c in range(NCH)}
            AV_AT = {}
            for g in range(NSLOT):
                AV_AT.setdefault(g + AV_LAG[g // JT], []).append(g)

            # ---- prologue: first k/q projections ----
            for part in (0, 1):
                proj_mm_half("k", 0, 0, kT[0][:, 0:512], part)
            for part in (0, 1):
                proj_mm_half("q", 0, 0, qT[0][:, 0:512], part)
            emit_scores(0)
            emit_scores(1)

            # ---- the stream ----
            paccs = {}

            norm_pend = {}

            def run_avs(slot):
                # second normalize half from the previous chunk, one slot
                # later, so the vector burst at a chunk boundary is halved
                for ca, pacc in list(norm_pend.items()):
                    emit_normalize_h(ca, pacc, 1)
                    del norm_pend[ca]
                for ga in AV_AT.get(slot, ()):
                    ca = ga // JT
                    if ga % JT == 0:
                        paccs[ca] = [psA.tile([128, QC], F32, name=f"pacc{h}",
                                              tag="pacc") for h in range(2)]
                    emit_av(ga, paccs[ca])
                    if ga % JT == JT - 1:
                        pacc = paccs.pop(ca)
                        emit_normalize_h(ca, pacc, 0)
                        norm_pend[ca] = pacc

            for g in range(NSLOT):
                emit_exp(g)
                if g + 2 < NSLOT:
                    emit_scores(g + 2)
                for th in FOLD.get(g, ()):
                    th()
                run_avs(g)

            # ---- tail: trailing avs, then last chunk's output projection ----
            for slot in range(NSLOT, NSLOT + max(AV_LAG.values()) + 1):
                run_avs(slot)
            for tt in range(28, 32):
                emit_oproj(tt, use_act=True)
    nc.finalize()
    return nc


@functools.cache
def _nc_cached():
    return build_nc()


def _prep_in_maps(inputs):
    np16 = np.float16

    def xbm(a):
        # [S,B,D] -> [128, (b hf) a m]: tile-contiguous per partition
        xd = np.asarray(a, np.float32).transpose(2, 1, 0)        # [D, B, S]
        xd = xd.reshape(DT, 128, B, 4, 512).transpose(1, 2, 3, 0, 4)
        return np.ascontiguousarray(xd.reshape(128, B * 4 * DT * 512)).astype(np16)

    xq_h = xbm(inputs["query"])
    xk_h = xbm(inputs["key"])
    xv_h = xbm(inputs["value"])
    Wq, Wk, Wv, Wo = (np.asarray(inputs[k], np.float32) for k in ("Wq", "Wk", "Wv", "Wo"))

    def wpack(w):
        # [D, FPC] (t p)-major -> [p=128, t*m] contiguous per partition
        return np.ascontiguousarray(
            w.reshape(DT, 128, FPC).transpose(1, 0, 2).reshape(128, DT * FPC)
        ).astype(np16)

    in_maps = []
    for c in range(NC):
        sl = slice(c * FPC, (c + 1) * FPC)
        in_maps.append({
            "xq": xq_h, "xk": xk_h, "xv": xv_h,
            "wq": wpack(Wq[sl, :].T),
            "wk": wpack(Wk[sl, :].T),
            "wv": wpack(Wv[sl, :].T),
            "wo": np.ascontiguousarray(Wo[:, sl].T).astype(np16),
        })
    return in_maps


def kernel(query, key, value, Wq, bq, Wk, bk, Wv, bv, Wo, bo):
    in_maps = _prep_in_maps({"query": query, "key": key, "value": value,
                             "Wq": Wq, "Wk": Wk, "Wv": Wv, "Wo": Wo})
    nc = _nc_cached()
    res = run_bass_kernel_spmd(nc, in_maps, core_ids=list(range(NC)))
    acc = np.zeros((T, D), np.float32)
    for r in res.results:
        acc += r["out"].astype(np.float32)
    acc += np.asarray(bo, np.float32)[None, :]
    # batch-major tokens back to [S, B, D]
    out = acc.reshape(B, S, D).transpose(1, 0, 2)
    out = np.ascontiguousarray(out, np.float32)
    for bias in (bq, bk, bv):
        assert float(np.abs(np.asarray(bias)).max()) == 0.0, "nonzero qkv bias unsupported"
    return out
